# revision 7
# baseline (speedup 1.0000x reference)
"""Trainium2 Bass kernel for nn_NeuralDecisionTree.

Math (per sample b):
  h[b,f,i] = x[b,f] * W[i] + bias[f,i],   W = [1,2,3,4],
  bias[f,:] = cumsum([0, -sort(cut_points[f])])           (f=0..7, i=0..3)
  leaf[b, i0..i7] = prod_f h[b,f,i_f]                      (65536-wide kron)
  out[b,c] = sum_leaf leaf[b,leaf] * leaf_score[leaf,c]    (c=0..9)

Kernel strategy (pure batch-data-parallel over 8 cores, 256 rows each):
  Split features into A = kron(h0,h1,h2) [B,64] and Bv = kron(h3..h7) [B,1024].
  out[b,c] = sum_u A[b,u] * R[b,u,c],  R[b,:,:] = Bv[b,:] @ LSs  where
  LSs[v, c*64+u] = leaf_score[u*1024+v, c]  (prepped on host, replicated).
  Per 128-row tile: build A/Bv with per-partition-scalar multiplies (DVE/ACT),
  transpose Bv via TensorE (8x 128x128), contract with LSs on TensorE
  (fp32r, N>=256 so full rate), final A-weighted segmented reduce on DVE.
"""

import os
import sys

sys.path.insert(0, "/opt/trn_rl_repo")

import numpy as np

import concourse.bass as bass
from concourse import bacc
import concourse.mybir as mybir
import concourse.tile as tile
from concourse.bass_utils import run_bass_kernel_spmd

F32 = mybir.dt.float32
F32R = mybir.dt.float32r

N_CORES = 8
BATCH = 2048
ROWS_PER_CORE = BATCH // N_CORES  # 256
TILES_PER_CORE = ROWS_PER_CORE // 128  # 2
NF = 8          # features
NB = 4          # bins per feature (D+1)
NC_OUT = 10     # classes
U = 64          # kron(feat 0,1,2)
V = 1024        # kron(feat 3..7)
VCHUNKS = V // 128  # 8
NCOL = NC_OUT * U   # 640 columns of LSs, layout c*64+u
NHALF = NCOL // 2   # 320 (two PSUM tiles, both >=256 for fp32r full rate)

LAST_RESULT = None  # BassKernelResults of the most recent run (for test.py)


def _build_nc():
    nc = bacc.Bacc("TRN2", target_bir_lowering=False, num_devices=N_CORES)
    x_in = nc.declare_dram_parameter("x", [ROWS_PER_CORE, NF], F32, isOutput=False)
    wb_in = nc.declare_dram_parameter("wb", [128, NF * NB], F32, isOutput=False)
    bb_in = nc.declare_dram_parameter("bb", [128, NF * NB], F32, isOutput=False)
    ls_in = nc.declare_dram_parameter("ls", [128, VCHUNKS * NCOL], F32R, isOutput=False)
    id_in = nc.declare_dram_parameter("ident", [128, 128], F32R, isOutput=False)
    out_ext = nc.declare_dram_parameter("out", [ROWS_PER_CORE, NC_OUT], F32, isOutput=True)

    with tile.TileContext(nc) as tc:
        with (
            tc.tile_pool(name="consts", bufs=1) as consts,
            tc.tile_pool(name="work", bufs=2) as work,
            tc.tile_pool(name="bt", bufs=2) as btp,
            tc.tile_pool(name="tpsum", bufs=2, space="PSUM") as tpsum,
            tc.tile_pool(name="rpsum", bufs=2, space="PSUM") as rpsum,
        ):
            wb = consts.tile([128, NF * NB], F32)
            nc.sync.dma_start(out=wb[:], in_=wb_in[:])
            bb = consts.tile([128, NF * NB], F32)
            nc.sync.dma_start(out=bb[:], in_=bb_in[:])
            ident = consts.tile([128, 128], F32R)
            nc.sync.dma_start(out=ident[:], in_=id_in[:])
            ls = consts.tile([128, VCHUNKS * NCOL], F32R)
            nc.sync.dma_start(out=ls[:], in_=ls_in[:])

            for t in range(TILES_PER_CORE):
                xt = work.tile([128, NF], F32, tag="xt")
                nc.sync.dma_start(out=xt[:], in_=x_in[t * 128:(t + 1) * 128, :])

                # h[:, f*4+i] = x[:, f] * W[i] + bias[f, i]
                h = work.tile([128, NF * NB], F32, tag="h")
                xr = xt[:].unsqueeze(2).broadcast_to([128, NF, NB])
                nc.vector.tensor_mul(
                    h[:].rearrange("p (f i) -> p f i", f=NF),
                    xr,
                    wb[:].rearrange("p (f i) -> p f i", f=NF),
                )
                nc.vector.tensor_add(h[:], h[:], bb[:])

                def hcol(f, i):
                    return h[:, f * NB + i:f * NB + i + 1]

                # A = kron(h0, h1, h2): A[:, i0*16 + i1*4 + i2]
                a1 = work.tile([128, 16], F32, tag="a1")
                for i1 in range(4):
                    nc.vector.tensor_scalar_mul(
                        a1[:, i1 * 4:(i1 + 1) * 4], h[:, 2 * NB:2 * NB + 4], hcol(1, i1)
                    )
                a = work.tile([128, U], F32, tag="a")
                for i0 in range(4):
                    nc.vector.tensor_scalar_mul(
                        a[:, i0 * 16:(i0 + 1) * 16], a1[:], hcol(0, i0)
                    )

                # Bv = kron(h3..h7): Bv[:, i3*256 + i4*64 + i5*16 + i6*4 + i7]
                b1 = work.tile([128, 16], F32, tag="b1")
                for i6 in range(4):
                    nc.vector.tensor_scalar_mul(
                        b1[:, i6 * 4:(i6 + 1) * 4], h[:, 7 * NB:7 * NB + 4], hcol(6, i6)
                    )
                b2 = work.tile([128, 64], F32, tag="b2")
                for i5 in range(4):
                    nc.vector.tensor_scalar_mul(
                        b2[:, i5 * 16:(i5 + 1) * 16], b1[:], hcol(5, i5)
                    )
                b3 = work.tile([128, 256], F32, tag="b3")
                for i4 in range(4):
                    nc.scalar.mul(b3[:, i4 * 64:(i4 + 1) * 64], b2[:], hcol(4, i4))
                b4 = work.tile([128, V], F32R, tag="b4")
                for i3 in range(4):
                    nc.vector.tensor_scalar_mul(
                        b4[:, i3 * 256:(i3 + 1) * 256], b3[:], hcol(3, i3)
                    )

                # Transpose Bv -> BT (8 chunks of [128,128]) via TensorE
                bt = btp.tile([128, V], F32R, tag="btile")
                for k in range(VCHUNKS):
                    pt = tpsum.tile([128, 128], F32R, tag="tp")
                    nc.tensor.transpose(pt[:], b4[:, k * 128:(k + 1) * 128], ident[:])
                    nc.scalar.copy(bt[:, k * 128:(k + 1) * 128], pt[:])

                # R[b, c*64+u] = sum_v Bv[b,v] * LSs[v, c*64+u]  (fp32r matmuls)
                ps0 = rpsum.tile([128, NHALF], F32, tag="ps0")
                ps1 = rpsum.tile([128, NHALF], F32, tag="ps1")
                for k in range(VCHUNKS):
                    lhsT = bt[:, k * 128:(k + 1) * 128]
                    base = k * NCOL
                    nc.tensor.matmul(
                        ps0[:], lhsT, ls[:, base:base + NHALF],
                        start=(k == 0), stop=(k == VCHUNKS - 1),
                    )
                    nc.tensor.matmul(
                        ps1[:], lhsT, ls[:, base + NHALF:base + NCOL],
                        start=(k == 0), stop=(k == VCHUNKS - 1),
                    )

                # out[b, c] = sum_u A[b,u] * R[b, c*64+u]
                tt = work.tile([128, NCOL], F32, tag="tt")
                abc = a[:].unsqueeze(1).broadcast_to([128, NC_OUT // 2, U])
                nc.vector.tensor_mul(
                    tt[:, 0:NHALF].rearrange("p (c u) -> p c u", u=U),
                    ps0[:].rearrange("p (c u) -> p c u", u=U),
                    abc,
                )
                nc.vector.tensor_mul(
                    tt[:, NHALF:NCOL].rearrange("p (c u) -> p c u", u=U),
                    ps1[:].rearrange("p (c u) -> p c u", u=U),
                    abc,
                )
                ot = work.tile([128, NC_OUT], F32, tag="ot")
                nc.vector.reduce_sum(
                    ot[:],
                    tt[:].rearrange("p (c u) -> p c u", u=U),
                    axis=mybir.AxisListType.X,
                )
                nc.sync.dma_start(out=out_ext[t * 128:(t + 1) * 128, :], in_=ot[:])

    nc.compile()
    return nc


_NC_CACHE = None


def _install_profiling():
    """Register the axon NTFF profile hook that this image's `antenv` lacks,
    so run_bass_kernel_spmd(trace=True) can measure HW exec time."""
    import types
    import contextlib

    try:
        import antenv.axon_hooks  # noqa: F401
        return True
    except ImportError:
        pass
    try:
        from trn_agent_boot.trn_boot import _ntff_profile_via_ctypes
        import antenv

        hook = _ntff_profile_via_ctypes("/opt/axon/libaxon_pjrt.so")
        if hook is None:
            return False
        mod = types.ModuleType("antenv.axon_hooks")
        mod._hook = hook
        mod.set_axon_ntff_profile_hook = lambda h: setattr(mod, "_hook", h)
        mod.get_axon_ntff_profile_hook = lambda: mod._hook
        sys.modules["antenv.axon_hooks"] = mod
        antenv.axon_hooks = mod

        # Artifact upload reaches for a remote bucket; keep everything local.
        import concourse.bass_utils as bu

        bu.upload_artifacts = lambda tmpdir: "local://" + str(tmpdir)
        return True
    except Exception as e:  # pragma: no cover - best effort
        print(f"profiling hook install failed: {e!r}", file=sys.stderr)
        return False


def _to_fp32r(a):
    """Round fp32 to the PE's fp32r format: mantissa truncated to 11 bits (RNE)."""
    u = np.ascontiguousarray(np.asarray(a, np.float32)).view(np.uint32)
    low = u & np.uint32(0xFFF)
    base = u & np.uint32(0xFFFFF000)
    add = (low > 0x800) | ((low == 0x800) & (((u >> np.uint32(12)) & np.uint32(1)) == 1))
    out = base + np.where(add, np.uint32(0x1000), np.uint32(0))
    return out.view(np.float32)


def _host_prep(cut_points, leaf_score):
    W = np.arange(1.0, NB + 1.0, dtype=np.float32)               # [4]
    cp = np.sort(cut_points.astype(np.float32), axis=-1)          # [8,3]
    bias = np.cumsum(
        np.concatenate([np.zeros((NF, 1), np.float32), -cp], axis=1), axis=1
    )                                                             # [8,4]
    wb = np.tile(W[None, None, :], (128, NF, 1)).reshape(128, NF * NB)
    bb = np.tile(bias[None, :, :], (128, 1, 1)).reshape(128, NF * NB)
    # LSs[p, k, c, u] = leaf_score[u*1024 + k*128 + p, c]
    ls4 = leaf_score.astype(np.float32).reshape(U, VCHUNKS, 128, NC_OUT)
    lss = np.ascontiguousarray(ls4.transpose(2, 1, 3, 0)).reshape(128, VCHUNKS * NCOL)
    lss = _to_fp32r(lss)
    ident = np.eye(128, dtype=np.float32)
    return wb, bb, lss, ident


def kernel(x, cut_points, leaf_score):
    global _NC_CACHE, LAST_RESULT
    x = np.ascontiguousarray(x, dtype=np.float32)
    wb, bb, lss, ident = _host_prep(
        np.asarray(cut_points), np.asarray(leaf_score)
    )
    if _NC_CACHE is None:
        _NC_CACHE = _build_nc()
    nc = _NC_CACHE

    in_maps = []
    for i in range(N_CORES):
        in_maps.append({
            "x": np.ascontiguousarray(x[i * ROWS_PER_CORE:(i + 1) * ROWS_PER_CORE]),
            "wb": wb, "bb": bb, "ls": lss, "ident": ident,
        })
    trace = bool(os.environ.get("BASS_TRACE"))
    if trace:
        trace = _install_profiling()
    res = run_bass_kernel_spmd(nc, in_maps, list(range(N_CORES)), trace=trace)
    LAST_RESULT = res
    out = np.concatenate([res.results[i]["out"] for i in range(N_CORES)], axis=0)
    return out


if __name__ == "__main__":
    rng = np.random.default_rng(0)
    x = rng.standard_normal((BATCH, NF), dtype=np.float32)
    cut_points = rng.random((NF, 3), dtype=np.float32)
    leaf_score = rng.random((65536, NC_OUT), dtype=np.float32)
    out = kernel(x, cut_points, leaf_score)
    print(out.shape, out.dtype, out[:2])


# revision 8
# speedup vs baseline: 1.2814x; 1.2814x over previous
"""Trainium2 Bass kernel for nn_NeuralDecisionTree.

Math (per sample b):
  h[b,f,i] = x[b,f] * W[i] + bias[f,i],   W = [1,2,3,4],
  bias[f,:] = cumsum([0, -sort(cut_points[f])])           (f=0..7, i=0..3)
  leaf[b, i0..i7] = prod_f h[b,f,i_f]                      (65536-wide kron)
  out[b,c] = sum_leaf leaf[b,leaf] * leaf_score[leaf,c]    (c=0..9)

Kernel strategy (pure batch-data-parallel over 8 cores, 256 rows each):
  W is folded into leaf_score on the host (h' = x + bias/W;
  LS' = leaf_score * kron(W,..,W)), so the device math is
  out[b,c] = sum_u A[b,u] * R[b,u,c],  R[b,:,:] = Bv[b,:] @ LSs,
  A = kron(h'0..h'2) [B,64], Bv = kron(h'3..h'7) [B,1024],
  LSs[v, c*64+u] = LS'[u*1024+v, c]  (host-prepped, fp32r-rounded, replicated).
  Per 128-row tile: kron via broadcast tensor_tensor ops (DVE/ACT),
  transpose Bv via TensorE into 2 packed PSUM banks, contract with LSs on
  TensorE (fp32r), final A-weighted segmented reduce on DVE.
"""

import os
import sys

sys.path.insert(0, "/opt/trn_rl_repo")

import numpy as np

import concourse.bass as bass
from concourse import bacc
import concourse.mybir as mybir
import concourse.tile as tile
from concourse.bass_utils import run_bass_kernel_spmd

F32 = mybir.dt.float32
F32R = mybir.dt.float32r

N_CORES = 8
BATCH = 2048
ROWS_PER_CORE = BATCH // N_CORES  # 256
TILES_PER_CORE = ROWS_PER_CORE // 128  # 2
NF = 8          # features
NB = 4          # bins per feature (D+1)
NC_OUT = 10     # classes
U = 64          # kron(feat 0,1,2)
V = 1024        # kron(feat 3..7)
VCHUNKS = V // 128  # 8
NCOL = NC_OUT * U   # 640 columns of LSs, layout c*64+u
NHALF = NCOL // 2   # 320 (two PSUM tiles per chunk-matmul)
LSDMA = 4           # number of chunked ls DMAs (2 v-chunks each)

LAST_RESULT = None  # BassKernelResults of the most recent run (for test.py)


def _build_nc():
    nc = bacc.Bacc("TRN2", target_bir_lowering=False, debug=False,
                   num_devices=N_CORES)
    x_in = nc.declare_dram_parameter("x", [ROWS_PER_CORE, NF], F32, isOutput=False)
    bb_in = nc.declare_dram_parameter("bb", [128, NF * NB], F32, isOutput=False)
    ls_in = nc.declare_dram_parameter("ls", [128, VCHUNKS * NCOL], F32R, isOutput=False)
    id_in = nc.declare_dram_parameter("ident", [128, 128], F32R, isOutput=False)
    out_ext = nc.declare_dram_parameter("out", [ROWS_PER_CORE, NC_OUT], F32, isOutput=True)

    with tile.TileContext(nc) as tc:
        with (
            tc.tile_pool(name="consts", bufs=1) as consts,
            tc.tile_pool(name="work", bufs=2) as work,
            tc.tile_pool(name="bt", bufs=2) as btp,
            tc.tile_pool(name="tpsum", bufs=3, space="PSUM") as tpsum,
            tc.tile_pool(name="rpsum", bufs=2, space="PSUM") as rpsum,
        ):
            # Small inputs on the scalar HWDGE ring so they don't queue
            # behind the big ls stream (which goes on the sync ring).
            xa = consts.tile([128, TILES_PER_CORE * NF], F32)
            nc.scalar.dma_start(
                out=xa[:].rearrange("p (t f) -> p t f", f=NF),
                in_=x_in[:].rearrange("(t p) f -> p t f", p=128),
            )
            bb = consts.tile([128, NF * NB], F32)
            nc.scalar.dma_start(out=bb[:], in_=bb_in[:])
            ident = consts.tile([128, 128], F32R)
            nc.scalar.dma_start(out=ident[:], in_=id_in[:])

            lst = []
            for j in range(LSDMA):
                lsj = consts.tile([128, (VCHUNKS // LSDMA) * NCOL], F32R, tag=f"ls{j}")
                sl = bass.ts(j, (VCHUNKS // LSDMA) * NCOL)
                nc.sync.dma_start(out=lsj[:], in_=ls_in[:, sl])
                lst.append(lsj)

            def ls_chunk(k, half):
                j, r = divmod(k, VCHUNKS // LSDMA)
                base = r * NCOL + half * NHALF
                return lst[j][:, base:base + NHALF]

            oa = consts.tile([128, TILES_PER_CORE * NC_OUT], F32)

            def bcast0(ap, i, shape):
                return ap.unsqueeze(i).broadcast_to(shape)

            for t in range(TILES_PER_CORE):
                # h'[:, f*4+i] = x[:, f] + bias[f,i]/W[i]
                h = work.tile([128, NF * NB], F32, tag="h")
                nc.vector.tensor_add(
                    h[:].rearrange("p (f i) -> p f i", f=NF),
                    bcast0(xa[:, t * NF:(t + 1) * NF], 2, [128, NF, NB]),
                    bb[:].rearrange("p (f i) -> p f i", f=NF),
                )

                def hcols(f):
                    return h[:, f * NB:(f + 1) * NB]

                def kron_step(out_t, width, prev, f, engine="vector"):
                    # out[:, i*width+s] = prev[:, s] * h'[:, f*4+i]
                    nc.vector.tensor_mul(
                        out_t[:].rearrange("p (i s) -> p i s", i=NB),
                        bcast0(prev[:], 1, [128, NB, width]),
                        bcast0(hcols(f), 2, [128, NB, width]),
                    )

                # A = kron(h0, h1, h2): A[:, i0*16 + i1*4 + i2]
                a1 = work.tile([128, 16], F32, tag="a1")
                kron_step(a1, 4, hcols(2), 1)
                a = work.tile([128, U], F32, tag="a")
                kron_step(a, 16, a1, 0)

                # Bv = kron(h3..h7): Bv[:, i3*256 + i4*64 + i5*16 + i6*4 + i7]
                b1 = work.tile([128, 16], F32, tag="b1")
                kron_step(b1, 4, hcols(7), 6)
                b2 = work.tile([128, 64], F32, tag="b2")
                kron_step(b2, 16, b1, 5)
                b3 = work.tile([128, 256], F32, tag="b3")
                kron_step(b3, 64, b2, 4)
                b4 = work.tile([128, V], F32R, tag="b4")
                # last level split: halves on DVE, halves on ACT
                nc.vector.tensor_mul(
                    b4[:, 0:512].rearrange("p (i s) -> p i s", i=2),
                    bcast0(b3[:], 1, [128, 2, 256]),
                    bcast0(h[:, 3 * NB:3 * NB + 2], 2, [128, 2, 256]),
                )
                for j in range(2):
                    nc.scalar.mul(
                        b4[:, 512 + j * 256:512 + (j + 1) * 256], b3[:],
                        h[:, 3 * NB + 2 + j:3 * NB + 3 + j],
                    )

                # Transpose Bv -> BT via TensorE; 4 chunk-transposes per
                # PSUM bank, evacuated with one wide ACT copy each.
                bt = btp.tile([128, V], F32R, tag="btile")
                for q in range(2):
                    tp = tpsum.tile([128, 512], F32R, tag="tp")
                    for j in range(4):
                        k = q * 4 + j
                        nc.tensor.transpose(
                            tp[:, j * 128:(j + 1) * 128],
                            b4[:, k * 128:(k + 1) * 128], ident[:],
                        )
                    nc.scalar.copy(bt[:, q * 512:(q + 1) * 512], tp[:])

                # R[b, c*64+u] = sum_v Bv[b,v] * LSs[v, c*64+u]  (fp32r)
                ps0 = rpsum.tile([128, NHALF], F32, tag="ps0")
                ps1 = rpsum.tile([128, NHALF], F32, tag="ps1")
                for k in range(VCHUNKS):
                    lhsT = bt[:, k * 128:(k + 1) * 128]
                    nc.tensor.matmul(
                        ps0[:], lhsT, ls_chunk(k, 0),
                        start=(k == 0), stop=(k == VCHUNKS - 1),
                    )
                    nc.tensor.matmul(
                        ps1[:], lhsT, ls_chunk(k, 1),
                        start=(k == 0), stop=(k == VCHUNKS - 1),
                    )

                # out[b, c] = sum_u A[b,u] * R[b, c*64+u]
                tt = work.tile([128, NCOL], F32, tag="tt")
                abc = bcast0(a[:], 1, [128, NC_OUT // 2, U])
                nc.vector.tensor_mul(
                    tt[:, 0:NHALF].rearrange("p (c u) -> p c u", u=U),
                    ps0[:].rearrange("p (c u) -> p c u", u=U),
                    abc,
                )
                nc.vector.tensor_mul(
                    tt[:, NHALF:NCOL].rearrange("p (c u) -> p c u", u=U),
                    ps1[:].rearrange("p (c u) -> p c u", u=U),
                    abc,
                )
                nc.vector.reduce_sum(
                    oa[:, t * NC_OUT:(t + 1) * NC_OUT],
                    tt[:].rearrange("p (c u) -> p c u", u=U),
                    axis=mybir.AxisListType.X,
                )

            nc.scalar.dma_start(
                out=out_ext[:].rearrange("(t p) c -> p t c", p=128),
                in_=oa[:].rearrange("p (t c) -> p t c", c=NC_OUT),
            )

    nc.compile()
    return nc


_NC_CACHE = None


def _install_profiling():
    """Register the axon NTFF profile hook that this image's `antenv` lacks,
    so run_bass_kernel_spmd(trace=True) can measure HW exec time."""
    import types

    try:
        import antenv.axon_hooks  # noqa: F401
        return True
    except ImportError:
        pass
    try:
        from trn_agent_boot.trn_boot import _ntff_profile_via_ctypes
        import antenv

        hook = _ntff_profile_via_ctypes("/opt/axon/libaxon_pjrt.so")
        if hook is None:
            return False
        mod = types.ModuleType("antenv.axon_hooks")
        mod._hook = hook
        mod.set_axon_ntff_profile_hook = lambda h: setattr(mod, "_hook", h)
        mod.get_axon_ntff_profile_hook = lambda: mod._hook
        sys.modules["antenv.axon_hooks"] = mod
        antenv.axon_hooks = mod

        # Artifact upload reaches for a remote bucket; keep everything local.
        import concourse.bass_utils as bu

        bu.upload_artifacts = lambda tmpdir: "local://" + str(tmpdir)
        return True
    except Exception as e:  # pragma: no cover - best effort
        print(f"profiling hook install failed: {e!r}", file=sys.stderr)
        return False


def _to_fp32r(a):
    """Round fp32 to the PE's fp32r format: mantissa truncated to 11 bits (RNE)."""
    u = np.ascontiguousarray(np.asarray(a, np.float32)).view(np.uint32)
    low = u & np.uint32(0xFFF)
    base = u & np.uint32(0xFFFFF000)
    add = (low > 0x800) | ((low == 0x800) & (((u >> np.uint32(12)) & np.uint32(1)) == 1))
    out = base + np.where(add, np.uint32(0x1000), np.uint32(0))
    return out.view(np.float32)


def _host_prep(cut_points, leaf_score):
    W = np.arange(1.0, NB + 1.0, dtype=np.float32)               # [4]
    cp = np.sort(cut_points.astype(np.float32), axis=-1)          # [8,3]
    bias = np.cumsum(
        np.concatenate([np.zeros((NF, 1), np.float32), -cp], axis=1), axis=1
    )                                                             # [8,4]
    # W folded into leaf_score: h' = x + bias/W, LS' = LS * kron(W,...,W)
    bb = np.tile((bias / W[None, :])[None, :, :], (128, 1, 1)).reshape(128, NF * NB)
    wk = np.array([1.0], dtype=np.float64)
    for _ in range(NF):
        wk = np.kron(wk, W.astype(np.float64))                    # [65536]
    lsw = (leaf_score.astype(np.float64) * wk[:, None]).astype(np.float32)
    # LSs[p, k, c, u] = LS'[u*1024 + k*128 + p, c]
    ls4 = lsw.reshape(U, VCHUNKS, 128, NC_OUT)
    lss = np.ascontiguousarray(ls4.transpose(2, 1, 3, 0)).reshape(128, VCHUNKS * NCOL)
    lss = _to_fp32r(lss)
    ident = np.eye(128, dtype=np.float32)
    return bb, lss, ident


def kernel(x, cut_points, leaf_score):
    global _NC_CACHE, LAST_RESULT
    x = np.ascontiguousarray(x, dtype=np.float32)
    bb, lss, ident = _host_prep(np.asarray(cut_points), np.asarray(leaf_score))
    if _NC_CACHE is None:
        _NC_CACHE = _build_nc()
    nc = _NC_CACHE

    in_maps = []
    for i in range(N_CORES):
        in_maps.append({
            "x": np.ascontiguousarray(x[i * ROWS_PER_CORE:(i + 1) * ROWS_PER_CORE]),
            "bb": bb, "ls": lss, "ident": ident,
        })
    trace = bool(os.environ.get("BASS_TRACE"))
    if trace:
        trace = _install_profiling()
    res = run_bass_kernel_spmd(nc, in_maps, list(range(N_CORES)), trace=trace)
    LAST_RESULT = res
    out = np.concatenate([res.results[i]["out"] for i in range(N_CORES)], axis=0)
    return out


if __name__ == "__main__":
    rng = np.random.default_rng(0)
    x = rng.standard_normal((BATCH, NF), dtype=np.float32)
    cut_points = rng.random((NF, 3), dtype=np.float32)
    leaf_score = rng.random((65536, NC_OUT), dtype=np.float32)
    out = kernel(x, cut_points, leaf_score)
    print(out.shape, out.dtype, out[:2])


# revision 11
# speedup vs baseline: 1.4915x; 1.1640x over previous
"""Trainium2 Bass kernel for nn_NeuralDecisionTree.

Math (per sample b):
  h[b,f,i] = x[b,f] * W[i] + bias[f,i],   W = [1,2,3,4],
  bias[f,:] = cumsum([0, -sort(cut_points[f])])           (f=0..7, i=0..3)
  leaf[b, i0..i7] = prod_f h[b,f,i_f]                      (65536-wide kron)
  out[b,c] = sum_leaf leaf[b,leaf] * leaf_score[leaf,c]    (c=0..9)

Kernel strategy (pure batch-data-parallel over 8 cores, 256 rows each):
  W is folded into leaf_score on the host (h' = x + bias/W;
  LS' = leaf_score * kron(W,..,W)), so the device math is
  out[b,c] = sum_u A[b,u] * R[b,u,c],  R[b,:,:] = Bv[b,:] @ LSs,
  A = kron(h'0..h'2) [B,64], Bv = kron(h'3..h'7) [B,1024],
  LSs[v, c*64+u] = LS'[u*1024+v, c]  (host-prepped, fp32r-rounded, replicated).
  Per 128-row tile: kron via broadcast tensor_tensor ops (DVE/ACT),
  transpose Bv via TensorE into 2 packed PSUM banks, contract with LSs on
  TensorE (fp32r), final A-weighted segmented reduce on DVE.
"""

import os
import sys

sys.path.insert(0, "/opt/trn_rl_repo")

import numpy as np

import concourse.bass as bass
from concourse import bacc
import concourse.mybir as mybir
import concourse.tile as tile
from concourse.bass_utils import run_bass_kernel_spmd

F32 = mybir.dt.float32
F32R = mybir.dt.float32r

N_CORES = 8
BATCH = 2048
ROWS_PER_CORE = BATCH // N_CORES  # 256
TILES_PER_CORE = ROWS_PER_CORE // 128  # 2
NF = 8          # features
NB = 4          # bins per feature (D+1)
NC_OUT = 10     # classes
U = 64          # kron(feat 0,1,2)
V = 1024        # kron(feat 3..7)
VCHUNKS = V // 128  # 8
NCOL = NC_OUT * U   # 640 columns of LSs, layout c*64+u
NHALF = NCOL // 2   # 320 (two PSUM tiles per chunk-matmul)
LSDMA = 4           # number of chunked ls DMAs (2 v-chunks each)
HEADC = TILES_PER_CORE * NF + NF * NB + 128  # head blob cols: x | bias | identity

LAST_RESULT = None  # BassKernelResults of the most recent run (for test.py)


def _build_nc():
    nc = bacc.Bacc("TRN2", target_bir_lowering=False, debug=False,
                   num_devices=N_CORES)
    head_in = nc.declare_dram_parameter("head", [128, HEADC], F32R, isOutput=False)
    ls_in = nc.declare_dram_parameter("ls", [128, VCHUNKS * NCOL], F32R, isOutput=False)
    out_ext = nc.declare_dram_parameter("out", [ROWS_PER_CORE, NC_OUT], F32, isOutput=True)

    with tile.TileContext(nc) as tc:
        with (
            tc.tile_pool(name="consts", bufs=1) as consts,
            tc.tile_pool(name="work", bufs=2) as work,
            tc.tile_pool(name="bt", bufs=2) as btp,
            tc.tile_pool(name="tpsum", bufs=4, space="PSUM") as tpsum,
            tc.tile_pool(name="rpsum", bufs=2, space="PSUM") as rpsum,
        ):
            # One head DMA (x | bias | identity) issued FIRST on the same
            # HWDGE ring as the ls stream: FIFO guarantees it lands before
            # the 2.6MB ls flood instead of starving behind it.
            head = consts.tile([128, HEADC], F32R)
            nc.sync.dma_start(out=head[:], in_=head_in[:])
            xa = head[:, 0:TILES_PER_CORE * NF].bitcast(F32)
            bb = head[:, TILES_PER_CORE * NF:TILES_PER_CORE * NF + NF * NB].bitcast(F32)
            ident = head[:, HEADC - 128:HEADC]

            lst = []
            for j in range(LSDMA):
                lsj = consts.tile([128, (VCHUNKS // LSDMA) * NCOL], F32R, tag=f"ls{j}")
                sl = bass.ts(j, (VCHUNKS // LSDMA) * NCOL)
                nc.sync.dma_start(out=lsj[:], in_=ls_in[:, sl])
                lst.append(lsj)

            def ls_chunk(k, half):
                j, r = divmod(k, VCHUNKS // LSDMA)
                base = r * NCOL + half * NHALF
                return lst[j][:, base:base + NHALF]

            oa = consts.tile([128, TILES_PER_CORE * NC_OUT], F32)

            def bcast0(ap, i, shape):
                return ap.unsqueeze(i).broadcast_to(shape)

            bts, avs, pss = [], [], []
            for t in range(TILES_PER_CORE):
                # h'[:, f*4+i] = x[:, f] + bias[f,i]/W[i]
                h = work.tile([128, NF * NB], F32, tag="h")
                nc.vector.tensor_add(
                    h[:].rearrange("p (f i) -> p f i", f=NF),
                    bcast0(xa[:, t * NF:(t + 1) * NF], 2, [128, NF, NB]),
                    bb[:].rearrange("p (f i) -> p f i", f=NF),
                )

                def hcols(f):
                    return h[:, f * NB:(f + 1) * NB]

                def kron_step(out_t, width, prev, f):
                    # out[:, i*width+s] = prev[:, s] * h'[:, f*4+i]
                    nc.vector.tensor_mul(
                        out_t[:].rearrange("p (i s) -> p i s", i=NB),
                        bcast0(prev[:], 1, [128, NB, width]),
                        bcast0(hcols(f), 2, [128, NB, width]),
                    )

                # A = kron(h0, h1, h2): A[:, i0*16 + i1*4 + i2]
                a1 = work.tile([128, 16], F32, tag="a1")
                kron_step(a1, 4, hcols(2), 1)
                a = work.tile([128, U], F32, tag="a")
                kron_step(a, 16, a1, 0)
                avs.append(a)

                # Bv = kron(h3..h7): Bv[:, i3*256 + i4*64 + i5*16 + i6*4 + i7]
                b1 = work.tile([128, 16], F32, tag="b1")
                kron_step(b1, 4, hcols(7), 6)
                b2 = work.tile([128, 64], F32, tag="b2")
                kron_step(b2, 16, b1, 5)
                b3 = work.tile([128, 256], F32, tag="b3")
                kron_step(b3, 64, b2, 4)
                b4 = work.tile([128, V], F32R, tag="b4")
                # last level split: halves on DVE, halves on ACT
                nc.vector.tensor_mul(
                    b4[:, 0:512].rearrange("p (i s) -> p i s", i=2),
                    bcast0(b3[:], 1, [128, 2, 256]),
                    bcast0(h[:, 3 * NB:3 * NB + 2], 2, [128, 2, 256]),
                )
                for j in range(2):
                    nc.scalar.mul(
                        b4[:, 512 + j * 256:512 + (j + 1) * 256], b3[:],
                        h[:, 3 * NB + 2 + j:3 * NB + 3 + j],
                    )

                # Transpose Bv -> BT via TensorE; 4 chunk-transposes per
                # PSUM bank, evacuated with one wide ACT copy each.
                bt = btp.tile([128, V], F32R, tag="btile")
                for q in range(2):
                    tp = tpsum.tile([128, 512], F32R, tag="tp")
                    for j in range(4):
                        k = q * 4 + j
                        nc.tensor.transpose(
                            tp[:, j * 128:(j + 1) * 128],
                            b4[:, k * 128:(k + 1) * 128], ident[:],
                        )
                    nc.scalar.copy(bt[:, q * 512:(q + 1) * 512], tp[:])
                bts.append(bt)
                pss.append((
                    rpsum.tile([128, NHALF], F32, tag="ps0", name=f"ps0_{t}"),
                    rpsum.tile([128, NHALF], F32, tag="ps1", name=f"ps1_{t}"),
                ))

            # R[b, c*64+u] = sum_v Bv[b,v] * LSs[v, c*64+u]  (fp32r),
            # tiles interleaved per v-chunk so matmuls track the ls stream.
            for k in range(VCHUNKS):
                for t in range(TILES_PER_CORE):
                    lhsT = bts[t][:, k * 128:(k + 1) * 128]
                    for half in range(2):
                        nc.tensor.matmul(
                            pss[t][half][:], lhsT, ls_chunk(k, half),
                            start=(k == 0), stop=(k == VCHUNKS - 1),
                        )

            # out[b, c] = sum_u A[b,u] * R[b, c*64+u], per psum half
            for t in range(TILES_PER_CORE):
                abc = bcast0(avs[t][:], 1, [128, NC_OUT // 2, U])
                for half in range(2):
                    tt = work.tile([128, NHALF], F32, tag="tt")
                    nc.vector.tensor_mul(
                        tt[:].rearrange("p (c u) -> p c u", u=U),
                        pss[t][half][:].rearrange("p (c u) -> p c u", u=U),
                        abc,
                    )
                    nc.vector.reduce_sum(
                        oa[:, t * NC_OUT + half * 5:t * NC_OUT + (half + 1) * 5],
                        tt[:].rearrange("p (c u) -> p c u", u=U),
                        axis=mybir.AxisListType.X,
                    )

            nc.scalar.dma_start(
                out=out_ext[:].rearrange("(t p) c -> p t c", p=128),
                in_=oa[:].rearrange("p (t c) -> p t c", c=NC_OUT),
            )

    nc.compile()
    return nc


_NC_CACHE = None


def _install_profiling():
    """Register the axon NTFF profile hook that this image's `antenv` lacks,
    so run_bass_kernel_spmd(trace=True) can measure HW exec time."""
    import types

    try:
        import antenv.axon_hooks  # noqa: F401
        return True
    except ImportError:
        pass
    try:
        from trn_agent_boot.trn_boot import _ntff_profile_via_ctypes
        import antenv

        hook = _ntff_profile_via_ctypes("/opt/axon/libaxon_pjrt.so")
        if hook is None:
            return False
        mod = types.ModuleType("antenv.axon_hooks")
        mod._hook = hook
        mod.set_axon_ntff_profile_hook = lambda h: setattr(mod, "_hook", h)
        mod.get_axon_ntff_profile_hook = lambda: mod._hook
        sys.modules["antenv.axon_hooks"] = mod
        antenv.axon_hooks = mod

        # Artifact upload reaches for a remote bucket; keep everything local.
        import concourse.bass_utils as bu

        bu.upload_artifacts = lambda tmpdir: "local://" + str(tmpdir)
        return True
    except Exception as e:  # pragma: no cover - best effort
        print(f"profiling hook install failed: {e!r}", file=sys.stderr)
        return False


def _to_fp32r(a):
    """Round fp32 to the PE's fp32r format: mantissa truncated to 11 bits (RNE)."""
    u = np.ascontiguousarray(np.asarray(a, np.float32)).view(np.uint32)
    low = u & np.uint32(0xFFF)
    base = u & np.uint32(0xFFFFF000)
    add = (low > 0x800) | ((low == 0x800) & (((u >> np.uint32(12)) & np.uint32(1)) == 1))
    out = base + np.where(add, np.uint32(0x1000), np.uint32(0))
    return out.view(np.float32)


def _host_prep(cut_points, leaf_score):
    W = np.arange(1.0, NB + 1.0, dtype=np.float32)               # [4]
    cp = np.sort(cut_points.astype(np.float32), axis=-1)          # [8,3]
    bias = np.cumsum(
        np.concatenate([np.zeros((NF, 1), np.float32), -cp], axis=1), axis=1
    )                                                             # [8,4]
    # W folded into leaf_score: h' = x + bias/W, LS' = LS * kron(W,...,W)
    bb = np.tile((bias / W[None, :])[None, :, :], (128, 1, 1)).reshape(128, NF * NB)
    wk = np.array([1.0], dtype=np.float64)
    for _ in range(NF):
        wk = np.kron(wk, W.astype(np.float64))                    # [65536]
    lsw = (leaf_score.astype(np.float64) * wk[:, None]).astype(np.float32)
    # LSs[p, k, c, u] = LS'[u*1024 + k*128 + p, c]
    ls4 = lsw.reshape(U, VCHUNKS, 128, NC_OUT)
    lss = np.ascontiguousarray(ls4.transpose(2, 1, 3, 0)).reshape(128, VCHUNKS * NCOL)
    lss = _to_fp32r(lss)
    ident = np.eye(128, dtype=np.float32)
    return bb, lss, ident


def _make_head(x_shard, bb, ident):
    head = np.empty((128, HEADC), dtype=np.float32)
    nx = TILES_PER_CORE * NF
    head[:, 0:nx] = x_shard.reshape(TILES_PER_CORE, 128, NF).transpose(1, 0, 2).reshape(128, nx)
    head[:, nx:nx + NF * NB] = bb
    head[:, HEADC - 128:HEADC] = ident
    return head


def kernel(x, cut_points, leaf_score):
    global _NC_CACHE, LAST_RESULT
    x = np.ascontiguousarray(x, dtype=np.float32)
    bb, lss, ident = _host_prep(np.asarray(cut_points), np.asarray(leaf_score))
    if _NC_CACHE is None:
        _NC_CACHE = _build_nc()
    nc = _NC_CACHE

    in_maps = []
    for i in range(N_CORES):
        xs = x[i * ROWS_PER_CORE:(i + 1) * ROWS_PER_CORE]
        in_maps.append({"head": _make_head(xs, bb, ident), "ls": lss})
    trace = bool(os.environ.get("BASS_TRACE"))
    if trace:
        trace = _install_profiling()
    res = run_bass_kernel_spmd(nc, in_maps, list(range(N_CORES)), trace=trace)
    LAST_RESULT = res
    out = np.concatenate([res.results[i]["out"] for i in range(N_CORES)], axis=0)
    return out


if __name__ == "__main__":
    rng = np.random.default_rng(0)
    x = rng.standard_normal((BATCH, NF), dtype=np.float32)
    cut_points = rng.random((NF, 3), dtype=np.float32)
    leaf_score = rng.random((65536, NC_OUT), dtype=np.float32)
    out = kernel(x, cut_points, leaf_score)
    print(out.shape, out.dtype, out[:2])
